# revision 19
# baseline (speedup 1.0000x reference)
"""AugmentedLstm Trainium2 kernel.

Math (faithful to the reference):
  pi = x_t @ Wt + b ; ps = h @ Wt + b   (Wt = W_in.T, [D, 6H])
  g  = pi[:, :5H] + ps[:, :5H] = (x_t + h) @ W5 + 2*b5      (W5 = Wt[:, :5H])
  gates i,f,m,o,hw from g;  c = i*m + f*c ; out = o*tanh(c)
  h = hw*out + (1-hw)*pi6   with pi6 = x_t @ W6 + b6 (precomputable, batched)
Masking (out/mem zeroed past the sequence length) only affects outputs at
t >= len, which we zero on the host; outputs for t < len are bit-identical.

Device strategy (8 cores, data-parallel, 2 sequences/core):
  - pi6 precomputed for all t with a PE-efficient batched matmul -> DRAM.
  - Step loop runs over a padded domain Tp = ceil(T/R)*R (R=32); the
    trailing garbage steps come after all real timesteps and their outputs
    are discarded on the host, so they cannot affect real outputs.
  - Serial step loop: per step one W-stationary matmul set (80 matmuls of
    [128k x 128m] weight tiles vs [128, 2] rhs = u.T = (x_t+h).T),
    output lands H-on-partitions for cheap [128, 8]-shaped gate math.
  - All layouts are "T-orientation": partitions = h-position within a
    128-chunk, free cols = (chunk k in 4, batch b in 2).

Perf notes (measured on HW):
  - The per-step cost is dominated by LDWEIGHTS+MATMUL instruction issue
    (~41-46 ns per [128x128] stationary tile at free-dim 2, FWL enabled);
    85 matmul instructions/step is the floor for this data-parallel split.
  - All stationary operands (W5 tiles, bias, sel) are fp8e4 ("fp8a"): fp8
    FWL loads are faster, and mixing stationary dtypes within a step
    regressed the pipeline. rhs (u) stays bf16: e4m3 weights x bf16 u gives
    rel err ~1.4e-2 (< 2e-2 tolerance, deterministic inputs); full fp8
    (u too) measured 2.0-2.8e-2 and is rejected.
  - All 5 bias matmuls are emitted before any u-dependent matmul so the
    in-order PE queue executes them during the previous step's tail.
  - Step-period model (validated to a few %): period = u-dependent pairs +
    max(620ns tail, u-free cover). ACT instructions cost ~(N+352)/1.2 ns
    (~295-310 ns fixed even for tiny N), strict FIFO; DVE fp32
    tensor_tensor = (N+151)/0.96 ns.
  - tailv2: merge the per-h-chunk hw-gate tail into 1 ACT + 2 DVE ops.
    ~5040 vs ~5600 ns/step (the chunked ACTs serialized ~1.2us of FIFO).
  - tailv3 (default): split each step into two h-chunk halves with 4-group
    matmul emission [bias][H0-k01][H0-k23][H1-k01][H1-k23] and a per-half
    activation/DVE tail. H0's gates stop ~20 pairs early, so u_{t+1}[0:2]
    lands before the step's pairs end, and H1's tail (+~0.6us) lands long
    before the next step's k23 group consumes it -> the end-of-step stall
    is structurally gone, with ~0.6-0.8us of margin on every dependency.
    Requires psum2=True (else the next step's bias matmuls WAR-wait on
    this step's late H1 PSUM reads). Accumulation order per (G,ms) is
    still k0,k1,k2,k3 -> output bit-identical. Measured in-batch:
    4962 vs 6052 (tailv2) ns/step med. The cut_chain no-dependency floor
    the same day was ~4650-4880 ns/step.
  - bias2 (optional): merge the 5 bias-init matmuls into 2 (one per psum
    tile; stationary = 12/8-row slabs of b5k, rhs = 0/1 indicators). Only
    pays once the stall is gone (pairs 85 -> 82).
  - deep_bufs=True (4-deep cs pool, 3-deep cu pool) + R=32 measured best.
  - Cross-core gate-split (2-way m-split halving matmuls/step, pairwise
    remote_dma h-exchange) was designed and probed but is NOT viable on the
    axon PJRT stack: SWDGE remote desc-gen crashes the exec unit
    (NRT_EXEC_UNIT_UNRECOVERABLE 101), remote_dma hostgen fails at NEFF
    load, and CC collectives (which do work) have a ~10-20 us/step floor.
"""

import numpy as np
import ml_dtypes

H = 512
NG = 5          # gates
M5 = 20         # 5H / 128 m-chunks
KC = 4          # 512 / 128 k-chunks
BL = 2          # sequences per core
NCORES = 8
CW = BL * KC    # columns per step slice (= 8)

_CACHE = {}


def _build(T, R, iters=None, outer_reps=1, staggered=False, hints=(), cut_chain=False, deep_bufs=False, wdt="bf16", psum2=False, tailv2=False, tailv3=False, bias2=False):
    import concourse.bass as bass
    import concourse.mybir as mybir
    import concourse.tile as tile
    from concourse import bacc
    from concourse.bass import ds

    f32 = mybir.dt.float32
    bf16 = mybir.dt.bfloat16
    wdtype = mybir.dt.float8e4 if wdt in ("fp8", "fp8a") else mybir.dt.bfloat16
    bdtype = mybir.dt.float8e4 if wdt == "fp8a" else mybir.dt.bfloat16
    AF = mybir.ActivationFunctionType
    ALU = mybir.AluOpType

    CH = min(256, T)
    Tp = ((T + R - 1) // R) * R
    assert T % CH == 0

    nc = bacc.Bacc("TRN2", target_bir_lowering=False, debug=False,
                   num_devices=NCORES)
    xT = nc.dram_tensor("xT", [128, (Tp + R) * CW], f32, kind="ExternalInput")
    w5 = nc.dram_tensor("w5", [128, M5 * KC * 128], wdtype, kind="ExternalInput")
    w6 = nc.dram_tensor("w6", [128, KC * KC * 128], f32, kind="ExternalInput")
    b5k = nc.dram_tensor("b5k", [M5, 128], bdtype, kind="ExternalInput")
    sel = nc.dram_tensor("sel", [KC, CW], bdtype, kind="ExternalInput")
    if bias2:
        # indicator rhs for the 2 merged bias matmuls (see step loop)
        selA = nc.dram_tensor("selA", [3 * KC, 3 * KC * BL], bdtype,
                              kind="ExternalInput")
        selB = nc.dram_tensor("selB", [2 * KC, 2 * KC * BL], bdtype,
                              kind="ExternalInput")
    b6 = nc.dram_tensor("b6", [128, KC], f32, kind="ExternalInput")
    outT = nc.dram_tensor("outT", [128, Tp * CW], f32, kind="ExternalOutput")

    with tile.TileContext(nc) as tc:
        with (
            tc.tile_pool(name="const", bufs=1) as constp,
            tc.tile_pool(name="dram", bufs=1, space="DRAM") as dramp,
            tc.tile_pool(name="state", bufs=1) as statep,
        ):
            w5_sb = constp.tile([128, M5, KC, 128], wdtype)
            nc.sync.dma_start(w5_sb[:], w5[:])
            w6_sb = constp.tile([128, KC, KC, 128], f32)
            nc.sync.dma_start(w6_sb[:], w6[:])
            b5k_sb = []
            for G in range(NG):
                t_b5k = constp.tile([KC, 128], bdtype, name=f"b5k{G}")
                nc.sync.dma_start(t_b5k[:], b5k[KC * G:KC * G + KC, :])
                b5k_sb.append(t_b5k)
            sel_sb = constp.tile([KC, CW], bdtype)
            nc.sync.dma_start(sel_sb[:], sel[:])
            b6_sb = constp.tile([128, KC], f32)
            nc.sync.dma_start(b6_sb[:], b6[:])
            if bias2:
                # merged bias stationaries: rows of b5k are (G,ms) in order,
                # so gates 0-2 = rows 0:12, gates 3-4 = rows 12:20.
                b5kA_sb = constp.tile([3 * KC, 128], bdtype, name="b5kA")
                nc.sync.dma_start(b5kA_sb[:], b5k[0:3 * KC, :])
                b5kB_sb = constp.tile([2 * KC, 128], bdtype, name="b5kB")
                nc.sync.dma_start(b5kB_sb[:], b5k[3 * KC:NG * KC, :])
                selA_sb = constp.tile([3 * KC, 3 * KC * BL], bdtype,
                                      name="selA")
                nc.sync.dma_start(selA_sb[:], selA[:])
                selB_sb = constp.tile([2 * KC, 2 * KC * BL], bdtype,
                                      name="selB")
                nc.sync.dma_start(selB_sb[:], selB[:])

            pi6T = dramp.tile([128, Tp * CW], f32)

            # ---- Phase B: pi6 = x @ W6 + b6 for all t, batched over time ----
            with (
                tc.tile_pool(name="bx", bufs=2) as bxp,
                tc.tile_pool(name="bo", bufs=2) as bop,
                tc.tile_pool(name="bps", bufs=4, space="PSUM") as bpsp,
            ):
                for ct in range(T // CH):
                    xc = bxp.tile([128, CH, KC, BL], f32)
                    nc.sync.dma_start(xc[:], xT[:, ct * CH * CW:(ct + 1) * CH * CW])
                    ob = bop.tile([128, CH, KC, BL], f32)
                    for m in range(KC):
                        ps = bpsp.tile([128, CH, BL], f32, tag="bps")
                        for k in range(KC):
                            nc.tensor.matmul(ps[:], w6_sb[:, m, k, :],
                                             xc[:, :, k, :],
                                             start=(k == 0), stop=(k == KC - 1))
                        nc.scalar.activation(ob[:, :, m, :], ps[:], AF.Identity,
                                             bias=b6_sb[:, m:m + 1])
                    nc.sync.dma_start(pi6T[:, ct * CH * CW:(ct + 1) * CH * CW],
                                      ob[:])

            # ---- Phase C: the serial recurrence ----
            c_st = statep.tile([128, KC, BL], f32)
            nc.vector.memset(c_st[:], 0.0)
            u_st = statep.tile([128, KC, BL], bf16)
            x0 = statep.tile([128, KC, BL], f32)
            nc.sync.dma_start(x0[:], xT[:, 0:CW])
            nc.vector.tensor_copy(u_st[:], x0[:])  # h0 = 0 -> u0 = x0

            with (
                tc.tile_pool(name="cx", bufs=2) as cxp,
                tc.tile_pool(name="cp", bufs=2) as cpp,
                tc.tile_pool(name="cr", bufs=2) as crp,
                tc.tile_pool(name="cs", bufs=4 if deep_bufs else 2) as csp,
                tc.tile_pool(name="cu", bufs=3 if deep_bufs else 2) as cup,
                tc.tile_pool(name="cps", bufs=2 if psum2 else 1, space="PSUM") as cpsp,
            ):
                n_iter = (Tp // R) if iters is None else iters
                hint_e = tuple(getattr(mybir.EngineType, h) for h in hints)
                with tc.For_i(0, n_iter * outer_reps,
                              staggered_reset=staggered,
                              hint_engines=hint_e) as ii:
                    i = (ii % n_iter) if outer_reps > 1 else ii
                    xblk = cxp.tile([128, R, KC, BL], f32)
                    nc.sync.dma_start(xblk[:], xT[:, ds((i * R + 1) * CW, R * CW)])
                    pblk = cpp.tile([128, R, KC, BL], f32)
                    nc.sync.dma_start(pblk[:], pi6T[:, ds(i * R * CW, R * CW)])
                    ring = crp.tile([128, R, KC, BL], f32)
                    xp = cxp.tile([128, R, KC, BL], f32, tag="xp")
                    nc.vector.tensor_add(xp[:], xblk[:], pblk[:])

                    u_cur = u_st
                    for s in range(R):
                        # Two merged psum tiles (i,f,m | o,hw): 2 tags so a
                        # bufs=2 psum pool fits in 8 banks (psum2 mode), which
                        # lets step s+1's bias matmuls run during step s's
                        # activation reads (no bank WAR serialization).
                        psA = cpsp.tile([128, 3, KC, BL], f32, tag="gA",
                                        name="psA")
                        psB = cpsp.tile([128, 2, KC, BL], f32, tag="gB",
                                        name="psB")

                        def pslice(G):
                            return psA[:, G, :, :] if G < 3 else psB[:, G - 3, :, :]

                        # All bias matmuls first: they do not depend on u, so
                        # the in-order PE queue can execute them while the
                        # previous step's activation/vector tail is still
                        # producing u (tail filler).
                        if bias2:
                            # one init matmul per psum tile: out[m,(G,ms,b)] =
                            # sum_r b5k[r,m]*ind[r,(G,ms,b)] = 2b[G,ms,m]
                            nc.tensor.matmul(
                                psA[:], b5kA_sb[:], selA_sb[:],
                                start=True, stop=False, skip_group_check=True)
                            nc.tensor.matmul(
                                psB[:], b5kB_sb[:], selB_sb[:],
                                start=True, stop=False, skip_group_check=True)
                        else:
                            for G in range(NG):
                                nc.tensor.matmul(
                                    pslice(G), b5k_sb[G][:],
                                    sel_sb[:], start=True, stop=False,
                                    skip_group_check=True)
                        if tailv3:
                            # 4-group emission [H0·k01][H0·k23][H1·k01][H1·k23]
                            # + per-half tails: H0's gates stop at pair 45 so
                            # u_{t+1}[0:2] is ready BEFORE this step's pairs
                            # end, and H1's tail (+~0.6us) lands long before
                            # the next step's k23 group needs it. Removes the
                            # end-of-step stall entirely (needs psum2 so the
                            # next step's bias matmuls don't WAR-wait on this
                            # step's late H1 activation reads). Accumulation
                            # order per (G,ms) stays k0,k1,k2,k3 ->
                            # bit-identical output.
                            if cut_chain:
                                u_nxt = cup.tile([128, KC, BL], bf16, tag="u")
                            elif s == R - 1:
                                u_nxt = u_st
                            else:
                                u_nxt = cup.tile([128, KC, BL], bf16, tag="u")
                            for half in range(2):
                                h0 = 2 * half
                                for kg in range(2):
                                    for G in range(NG):
                                        for ms in (h0, h0 + 1):
                                            m = KC * G + ms
                                            for k in (2 * kg, 2 * kg + 1):
                                                nc.tensor.matmul(
                                                    pslice(G)[:, ms, :],
                                                    w5_sb[:, m, k, :],
                                                    u_cur[:, k, :],
                                                    start=False,
                                                    stop=(k == KC - 1),
                                                    skip_group_check=True)
                                hsl = slice(h0, h0 + 2)
                                sif = csp.tile([128, 2, 2, BL], f32,
                                               tag=f"sif{half}")
                                mg = csp.tile([128, 2, BL], f32,
                                              tag=f"mg{half}")
                                og = csp.tile([128, 2, BL], f32,
                                              tag=f"og{half}")
                                hg = csp.tile([128, 2, BL], f32,
                                              tag=f"hg{half}")
                                nc.scalar.activation(sif[:],
                                                     psA[:, 0:2, hsl, :],
                                                     AF.Sigmoid)
                                nc.scalar.activation(mg[:], psA[:, 2, hsl, :],
                                                     AF.Tanh)
                                nc.scalar.activation(og[:], psB[:, 0, hsl, :],
                                                     AF.Sigmoid)
                                t1 = csp.tile([128, 2, BL], f32,
                                              tag=f"t1{half}")
                                nc.vector.tensor_mul(t1[:], sif[:, 0, :, :],
                                                     mg[:])
                                t2 = csp.tile([128, 2, BL], f32,
                                              tag=f"t2{half}")
                                nc.vector.tensor_mul(t2[:], sif[:, 1, :, :],
                                                     c_st[:, hsl, :])
                                nc.vector.tensor_add(c_st[:, hsl, :], t1[:],
                                                     t2[:])
                                tch = csp.tile([128, 2, BL], f32,
                                               tag=f"tch{half}")
                                nc.scalar.activation(tch[:], c_st[:, hsl, :],
                                                     AF.Tanh)
                                opv = csp.tile([128, 2, BL], f32,
                                               tag=f"opv{half}")
                                nc.vector.tensor_mul(opv[:], og[:], tch[:])
                                dv = csp.tile([128, 2, BL], f32,
                                              tag=f"dv{half}")
                                nc.vector.tensor_sub(dv[:], opv[:],
                                                     pblk[:, s, hsl, :])
                                nc.scalar.activation(hg[:], psB[:, 1, hsl, :],
                                                     AF.Sigmoid)
                                ev = csp.tile([128, 2, BL], f32,
                                              tag=f"ev{half}")
                                nc.vector.tensor_mul(ev[:], hg[:], dv[:])
                                nc.vector.tensor_add(u_nxt[:, hsl, :],
                                                     xp[:, s, hsl, :], ev[:])
                                nc.vector.tensor_add(ring[:, s, hsl, :],
                                                     ev[:],
                                                     pblk[:, s, hsl, :])
                            u_cur = u_st if cut_chain else u_nxt
                            continue
                        for G in range(NG):
                            for ms in range(KC):
                                m = KC * G + ms
                                for k in range(KC):
                                    nc.tensor.matmul(
                                        pslice(G)[:, ms, :],
                                        w5_sb[:, m, k, :],
                                        u_cur[:, k, :],
                                        start=False, stop=(k == KC - 1),
                                        skip_group_check=True)
                        sif = csp.tile([128, 2, KC, BL], f32, tag="sif")
                        mg = csp.tile([128, KC, BL], f32, tag="mg")
                        og = csp.tile([128, KC, BL], f32, tag="og")
                        hg = csp.tile([128, KC, BL], f32, tag="hg")
                        nc.scalar.activation(sif[:], psA[:, 0:2, :, :], AF.Sigmoid)
                        nc.scalar.activation(mg[:], psA[:, 2, :, :], AF.Tanh)
                        nc.scalar.activation(og[:], psB[:, 0, :, :], AF.Sigmoid)
                        t1 = csp.tile([128, KC, BL], f32, tag="t1")
                        nc.vector.tensor_mul(t1[:], sif[:, 0, :, :], mg[:])
                        t2 = csp.tile([128, KC, BL], f32, tag="t2")
                        nc.vector.tensor_mul(t2[:], sif[:, 1, :, :], c_st[:])
                        nc.vector.tensor_add(c_st[:], t1[:], t2[:])
                        tch = csp.tile([128, KC, BL], f32, tag="tch")
                        nc.scalar.activation(tch[:], c_st[:], AF.Tanh)
                        opv = csp.tile([128, KC, BL], f32, tag="opv")
                        nc.vector.tensor_mul(opv[:], og[:], tch[:])
                        dv = csp.tile([128, KC, BL], f32, tag="dv")
                        nc.vector.tensor_sub(dv[:], opv[:], pblk[:, s, :, :])
                        ev = csp.tile([128, KC, BL], f32, tag="ev")
                        if cut_chain:
                            u_nxt = cup.tile([128, KC, BL], bf16, tag="u")
                        elif s == R - 1:
                            u_nxt = u_st
                        else:
                            u_nxt = cup.tile([128, KC, BL], bf16, tag="u")
                        if tailv2:
                            # merged tail: one ACT + two DVE ops instead of
                            # 4x(ACT+2 DVE) — cuts end-of-step ACT FIFO
                            # congestion (each ACT costs ~(N+352)/1.2 ns).
                            nc.scalar.activation(hg[:], psB[:, 1, :, :],
                                                 AF.Sigmoid)
                            nc.vector.tensor_mul(ev[:], hg[:], dv[:])
                            nc.vector.tensor_add(u_nxt[:], xp[:, s, :, :],
                                                 ev[:])
                        else:
                            # hw-gate sigmoid, ev, and u are chunked per
                            # h-chunk so u[:, 0, :] lands as soon as psg4's
                            # ms=0 column stops; the next step's k=0 matmuls
                            # start while ms=1..3 of this step still drain
                            # through the PE.
                            for ms in range(KC):
                                nc.scalar.activation(hg[:, ms, :],
                                                     psB[:, 1, ms, :],
                                                     AF.Sigmoid)
                                nc.vector.tensor_mul(ev[:, ms, :],
                                                     hg[:, ms, :],
                                                     dv[:, ms, :])
                                nc.vector.tensor_add(u_nxt[:, ms, :],
                                                     xp[:, s, ms, :],
                                                     ev[:, ms, :])
                        nc.vector.tensor_add(ring[:, s, :, :], ev[:],
                                             pblk[:, s, :, :])
                        u_cur = u_st if cut_chain else u_nxt

                    nc.sync.dma_start(outT[:, ds(i * R * CW, R * CW)], ring[:])

    nc.compile()
    return nc


def _get_module(T, R, iters=None, outer_reps=1, staggered=False, hints=(),
                cut_chain=False, deep_bufs=False, wdt="bf16", psum2=False,
                tailv2=False, tailv3=False, bias2=False):
    key = (T, R, iters, outer_reps, staggered, tuple(hints), cut_chain,
           deep_bufs, wdt, psum2, tailv2, tailv3, bias2)
    if key not in _CACHE:
        _CACHE[key] = _build(T, R, iters, outer_reps, staggered, hints,
                             cut_chain, deep_bufs, wdt, psum2, tailv2, tailv3,
                             bias2)
    return _CACHE[key]


def _make_in_maps(x, W_in, b_in, R, wdt="bf16"):
    B, T, D = x.shape
    Tp = ((T + R - 1) // R) * R
    Wt = W_in.T  # [D, 6H]
    W5 = Wt[:, :NG * H]
    W6 = Wt[:, NG * H:]
    w5_np = ml_dtypes.float8_e4m3fn if wdt in ("fp8", "fp8a") else ml_dtypes.bfloat16
    b_np = ml_dtypes.float8_e4m3fn if wdt == "fp8a" else ml_dtypes.bfloat16
    w5_arr = np.ascontiguousarray(
        W5.reshape(KC, 128, M5, 128).transpose(1, 2, 0, 3)
        .reshape(128, M5 * KC * 128)).astype(w5_np)
    w6_arr = np.ascontiguousarray(
        W6.reshape(KC, 128, KC, 128).transpose(1, 2, 0, 3)
        .reshape(128, KC * KC * 128)).astype(np.float32)
    b5k_arr = np.ascontiguousarray((2.0 * b_in[:NG * H]).reshape(M5, 128)
                                   ).astype(b_np)
    sel_arr = np.zeros((KC, CW), b_np)
    for k in range(KC):
        sel_arr[k, BL * k:BL * k + BL] = 1.0
    # merged-bias indicators (bias2): rows (g,ms) -> cols (g,ms,b)
    selA_arr = np.zeros((3 * KC, 3 * KC * BL), b_np)
    for r in range(3 * KC):
        selA_arr[r, BL * r:BL * r + BL] = 1.0
    selB_arr = np.zeros((2 * KC, 2 * KC * BL), b_np)
    for r in range(2 * KC):
        selB_arr[r, BL * r:BL * r + BL] = 1.0
    b6_arr = np.ascontiguousarray(b_in[NG * H:].reshape(KC, 128).T
                                  ).astype(np.float32)
    in_maps = []
    for c in range(NCORES):
        xs = x[BL * c:BL * (c + 1)]  # [BL, T, D]
        xTa = np.zeros((128, (Tp + R) * CW), np.float32)
        xTa[:, :T * CW] = (xs.reshape(BL, T, KC, 128).transpose(3, 1, 2, 0)
                           .reshape(128, T * CW))
        in_maps.append({"xT": xTa, "w5": w5_arr, "w6": w6_arr,
                        "b5k": b5k_arr, "sel": sel_arr, "b6": b6_arr,
                        "selA": selA_arr, "selB": selB_arr})
    return in_maps


WDT = "fp8a"
# Best measured configuration (see perf notes): tailv3 = half-split step with
# 4-group matmul emission + per-half tails + double-buffered PSUM; removes
# the end-of-step dependency stall entirely (measured 4962 vs 6052 ns/step
# in-batch vs tailv2, bit-identical output).
BEST = dict(deep_bufs=True, tailv3=True, psum2=True, bias2=True)


def kernel(x, lengths, W_in, b_in):
    from concourse import bass_utils

    x = np.asarray(x, dtype=np.float32)
    lengths = np.asarray(lengths).astype(np.int64)
    W_in = np.asarray(W_in, dtype=np.float32)
    b_in = np.asarray(b_in, dtype=np.float32)
    B, T, D = x.shape
    R = 32
    nc = _get_module(T, R, wdt=WDT, **BEST)
    in_maps = _make_in_maps(x, W_in, b_in, R, wdt=WDT)
    res = bass_utils.run_bass_kernel_spmd(nc, in_maps,
                                          core_ids=list(range(NCORES)))
    out = np.zeros((B, T, D), np.float32)
    Tp = ((T + R - 1) // R) * R
    for c in range(NCORES):
        oT = np.asarray(res.results[c]["outT"])[:, :T * CW]
        oc = (oT.reshape(128, T, KC, BL).transpose(3, 1, 2, 0)
              .reshape(BL, T, D))
        out[BL * c:BL * (c + 1)] = oc
    mask = np.arange(T)[None, :] < lengths[:, None]
    out *= mask[:, :, None].astype(np.float32)
    return out



# revision 20
# speedup vs baseline: 1.2211x; 1.2211x over previous
"""AugmentedLstm Trainium2 kernel.

Math (faithful to the reference):
  pi = x_t @ Wt + b ; ps = h @ Wt + b   (Wt = W_in.T, [D, 6H])
  g  = pi[:, :5H] + ps[:, :5H] = (x_t + h) @ W5 + 2*b5      (W5 = Wt[:, :5H])
  gates i,f,m,o,hw from g;  c = i*m + f*c ; out = o*tanh(c)
  h = hw*out + (1-hw)*pi6   with pi6 = x_t @ W6 + b6 (precomputable, batched)
Masking (out/mem zeroed past the sequence length) only affects outputs at
t >= len, which we zero on the host; outputs for t < len are bit-identical.

Device strategy (8 cores, data-parallel, 2 sequences/core):
  - pi6 precomputed for all t with a PE-efficient batched matmul -> DRAM.
  - Step loop runs over a padded domain Tp = ceil(T/R)*R (R=32); the
    trailing garbage steps come after all real timesteps and their outputs
    are discarded on the host, so they cannot affect real outputs.
  - Serial step loop: per step one W-stationary matmul set (80 matmuls of
    [128k x 128m] weight tiles vs [128, 2] rhs = u.T = (x_t+h).T),
    output lands H-on-partitions for cheap [128, 8]-shaped gate math.
  - All layouts are "T-orientation": partitions = h-position within a
    128-chunk, free cols = (chunk k in 4, batch b in 2).

Perf notes (measured on HW):
  - The per-step cost is dominated by LDWEIGHTS+MATMUL instruction issue
    (~41-46 ns per [128x128] stationary tile at free-dim 2, FWL enabled);
    85 matmul instructions/step is the floor for this data-parallel split.
  - All stationary operands (W5 tiles, bias, sel) are fp8e4 ("fp8a"): fp8
    FWL loads are faster, and mixing stationary dtypes within a step
    regressed the pipeline. rhs (u) stays bf16: e4m3 weights x bf16 u gives
    rel err ~1.4e-2 (< 2e-2 tolerance, deterministic inputs); full fp8
    (u too) measured 2.0-2.8e-2 and is rejected.
  - All 5 bias matmuls are emitted before any u-dependent matmul so the
    in-order PE queue executes them during the previous step's tail.
  - Step-period model (validated to a few %): period = u-dependent pairs +
    max(620ns tail, u-free cover). ACT instructions cost ~(N+352)/1.2 ns
    (~295-310 ns fixed even for tiny N), strict FIFO; DVE fp32
    tensor_tensor = (N+151)/0.96 ns.
  - tailv2: merge the per-h-chunk hw-gate tail into 1 ACT + 2 DVE ops.
    ~5040 vs ~5600 ns/step (the chunked ACTs serialized ~1.2us of FIFO).
  - tailv3 (default): split each step into two h-chunk halves with 4-group
    matmul emission [bias][H0-k01][H0-k23][H1-k01][H1-k23] and a per-half
    activation/DVE tail. H0's gates stop ~20 pairs early, so u_{t+1}[0:2]
    lands before the step's pairs end, and H1's tail (+~0.6us) lands long
    before the next step's k23 group consumes it -> the end-of-step stall
    is structurally gone, with ~0.6-0.8us of margin on every dependency.
    Requires psum2=True (else the next step's bias matmuls WAR-wait on
    this step's late H1 PSUM reads). Accumulation order per (G,ms) is
    still k0,k1,k2,k3 -> output bit-identical. Measured in-batch:
    4962 vs 6052 (tailv2) ns/step med. The cut_chain no-dependency floor
    the same day was ~4650-4880 ns/step.
  - bias2 (optional, OFF): merge the 5 bias-init matmuls into 2. Measured
    +2.3% then -2.5% in consecutive in-batch A/Bs (= noise; the bias pairs
    are u-free cover, so removing them is period-neutral whenever any
    residual tail stall exists) and perturbs numerics by 2.6e-3. Left
    available but not enabled.
  - deep_bufs=True (4-deep cs pool, 3-deep cu pool) + R=32 measured best.
  - Cross-core gate-split (2-way m-split halving matmuls/step, pairwise
    remote_dma h-exchange) was designed and probed but is NOT viable on the
    axon PJRT stack: SWDGE remote desc-gen crashes the exec unit
    (NRT_EXEC_UNIT_UNRECOVERABLE 101), remote_dma hostgen fails at NEFF
    load, and CC collectives (which do work) have a ~10-20 us/step floor.
"""

import numpy as np
import ml_dtypes

H = 512
NG = 5          # gates
M5 = 20         # 5H / 128 m-chunks
KC = 4          # 512 / 128 k-chunks
BL = 2          # sequences per core
NCORES = 8
CW = BL * KC    # columns per step slice (= 8)

_CACHE = {}


def _build(T, R, iters=None, outer_reps=1, staggered=False, hints=(), cut_chain=False, deep_bufs=False, wdt="bf16", psum2=False, tailv2=False, tailv3=False, bias2=False):
    import concourse.bass as bass
    import concourse.mybir as mybir
    import concourse.tile as tile
    from concourse import bacc
    from concourse.bass import ds

    f32 = mybir.dt.float32
    bf16 = mybir.dt.bfloat16
    wdtype = mybir.dt.float8e4 if wdt in ("fp8", "fp8a") else mybir.dt.bfloat16
    bdtype = mybir.dt.float8e4 if wdt == "fp8a" else mybir.dt.bfloat16
    AF = mybir.ActivationFunctionType
    ALU = mybir.AluOpType

    CH = min(256, T)
    Tp = ((T + R - 1) // R) * R
    assert T % CH == 0

    nc = bacc.Bacc("TRN2", target_bir_lowering=False, debug=False,
                   num_devices=NCORES)
    xT = nc.dram_tensor("xT", [128, (Tp + R) * CW], f32, kind="ExternalInput")
    w5 = nc.dram_tensor("w5", [128, M5 * KC * 128], wdtype, kind="ExternalInput")
    w6 = nc.dram_tensor("w6", [128, KC * KC * 128], f32, kind="ExternalInput")
    b5k = nc.dram_tensor("b5k", [M5, 128], bdtype, kind="ExternalInput")
    sel = nc.dram_tensor("sel", [KC, CW], bdtype, kind="ExternalInput")
    if bias2:
        # indicator rhs for the 2 merged bias matmuls (see step loop)
        selA = nc.dram_tensor("selA", [3 * KC, 3 * KC * BL], bdtype,
                              kind="ExternalInput")
        selB = nc.dram_tensor("selB", [2 * KC, 2 * KC * BL], bdtype,
                              kind="ExternalInput")
    b6 = nc.dram_tensor("b6", [128, KC], f32, kind="ExternalInput")
    outT = nc.dram_tensor("outT", [128, Tp * CW], f32, kind="ExternalOutput")

    with tile.TileContext(nc) as tc:
        with (
            tc.tile_pool(name="const", bufs=1) as constp,
            tc.tile_pool(name="dram", bufs=1, space="DRAM") as dramp,
            tc.tile_pool(name="state", bufs=1) as statep,
        ):
            w5_sb = constp.tile([128, M5, KC, 128], wdtype)
            nc.sync.dma_start(w5_sb[:], w5[:])
            w6_sb = constp.tile([128, KC, KC, 128], f32)
            nc.sync.dma_start(w6_sb[:], w6[:])
            b5k_sb = []
            for G in range(NG):
                t_b5k = constp.tile([KC, 128], bdtype, name=f"b5k{G}")
                nc.sync.dma_start(t_b5k[:], b5k[KC * G:KC * G + KC, :])
                b5k_sb.append(t_b5k)
            sel_sb = constp.tile([KC, CW], bdtype)
            nc.sync.dma_start(sel_sb[:], sel[:])
            b6_sb = constp.tile([128, KC], f32)
            nc.sync.dma_start(b6_sb[:], b6[:])
            if bias2:
                # merged bias stationaries: rows of b5k are (G,ms) in order,
                # so gates 0-2 = rows 0:12, gates 3-4 = rows 12:20.
                b5kA_sb = constp.tile([3 * KC, 128], bdtype, name="b5kA")
                nc.sync.dma_start(b5kA_sb[:], b5k[0:3 * KC, :])
                b5kB_sb = constp.tile([2 * KC, 128], bdtype, name="b5kB")
                nc.sync.dma_start(b5kB_sb[:], b5k[3 * KC:NG * KC, :])
                selA_sb = constp.tile([3 * KC, 3 * KC * BL], bdtype,
                                      name="selA")
                nc.sync.dma_start(selA_sb[:], selA[:])
                selB_sb = constp.tile([2 * KC, 2 * KC * BL], bdtype,
                                      name="selB")
                nc.sync.dma_start(selB_sb[:], selB[:])

            pi6T = dramp.tile([128, Tp * CW], f32)

            # ---- Phase B: pi6 = x @ W6 + b6 for all t, batched over time ----
            with (
                tc.tile_pool(name="bx", bufs=2) as bxp,
                tc.tile_pool(name="bo", bufs=2) as bop,
                tc.tile_pool(name="bps", bufs=4, space="PSUM") as bpsp,
            ):
                for ct in range(T // CH):
                    xc = bxp.tile([128, CH, KC, BL], f32)
                    nc.sync.dma_start(xc[:], xT[:, ct * CH * CW:(ct + 1) * CH * CW])
                    ob = bop.tile([128, CH, KC, BL], f32)
                    for m in range(KC):
                        ps = bpsp.tile([128, CH, BL], f32, tag="bps")
                        for k in range(KC):
                            nc.tensor.matmul(ps[:], w6_sb[:, m, k, :],
                                             xc[:, :, k, :],
                                             start=(k == 0), stop=(k == KC - 1))
                        nc.scalar.activation(ob[:, :, m, :], ps[:], AF.Identity,
                                             bias=b6_sb[:, m:m + 1])
                    nc.sync.dma_start(pi6T[:, ct * CH * CW:(ct + 1) * CH * CW],
                                      ob[:])

            # ---- Phase C: the serial recurrence ----
            c_st = statep.tile([128, KC, BL], f32)
            nc.vector.memset(c_st[:], 0.0)
            u_st = statep.tile([128, KC, BL], bf16)
            x0 = statep.tile([128, KC, BL], f32)
            nc.sync.dma_start(x0[:], xT[:, 0:CW])
            nc.vector.tensor_copy(u_st[:], x0[:])  # h0 = 0 -> u0 = x0

            with (
                tc.tile_pool(name="cx", bufs=2) as cxp,
                tc.tile_pool(name="cp", bufs=2) as cpp,
                tc.tile_pool(name="cr", bufs=2) as crp,
                tc.tile_pool(name="cs", bufs=4 if deep_bufs else 2) as csp,
                tc.tile_pool(name="cu", bufs=3 if deep_bufs else 2) as cup,
                tc.tile_pool(name="cps", bufs=2 if psum2 else 1, space="PSUM") as cpsp,
            ):
                n_iter = (Tp // R) if iters is None else iters
                hint_e = tuple(getattr(mybir.EngineType, h) for h in hints)
                with tc.For_i(0, n_iter * outer_reps,
                              staggered_reset=staggered,
                              hint_engines=hint_e) as ii:
                    i = (ii % n_iter) if outer_reps > 1 else ii
                    xblk = cxp.tile([128, R, KC, BL], f32)
                    nc.sync.dma_start(xblk[:], xT[:, ds((i * R + 1) * CW, R * CW)])
                    pblk = cpp.tile([128, R, KC, BL], f32)
                    nc.sync.dma_start(pblk[:], pi6T[:, ds(i * R * CW, R * CW)])
                    ring = crp.tile([128, R, KC, BL], f32)
                    xp = cxp.tile([128, R, KC, BL], f32, tag="xp")
                    nc.vector.tensor_add(xp[:], xblk[:], pblk[:])

                    u_cur = u_st
                    for s in range(R):
                        # Two merged psum tiles (i,f,m | o,hw): 2 tags so a
                        # bufs=2 psum pool fits in 8 banks (psum2 mode), which
                        # lets step s+1's bias matmuls run during step s's
                        # activation reads (no bank WAR serialization).
                        psA = cpsp.tile([128, 3, KC, BL], f32, tag="gA",
                                        name="psA")
                        psB = cpsp.tile([128, 2, KC, BL], f32, tag="gB",
                                        name="psB")

                        def pslice(G):
                            return psA[:, G, :, :] if G < 3 else psB[:, G - 3, :, :]

                        # All bias matmuls first: they do not depend on u, so
                        # the in-order PE queue can execute them while the
                        # previous step's activation/vector tail is still
                        # producing u (tail filler).
                        if bias2:
                            # one init matmul per psum tile: out[m,(G,ms,b)] =
                            # sum_r b5k[r,m]*ind[r,(G,ms,b)] = 2b[G,ms,m]
                            nc.tensor.matmul(
                                psA[:], b5kA_sb[:], selA_sb[:],
                                start=True, stop=False, skip_group_check=True)
                            nc.tensor.matmul(
                                psB[:], b5kB_sb[:], selB_sb[:],
                                start=True, stop=False, skip_group_check=True)
                        else:
                            for G in range(NG):
                                nc.tensor.matmul(
                                    pslice(G), b5k_sb[G][:],
                                    sel_sb[:], start=True, stop=False,
                                    skip_group_check=True)
                        if tailv3:
                            # 4-group emission [H0·k01][H0·k23][H1·k01][H1·k23]
                            # + per-half tails: H0's gates stop at pair 45 so
                            # u_{t+1}[0:2] is ready BEFORE this step's pairs
                            # end, and H1's tail (+~0.6us) lands long before
                            # the next step's k23 group needs it. Removes the
                            # end-of-step stall entirely (needs psum2 so the
                            # next step's bias matmuls don't WAR-wait on this
                            # step's late H1 activation reads). Accumulation
                            # order per (G,ms) stays k0,k1,k2,k3 ->
                            # bit-identical output.
                            if cut_chain:
                                u_nxt = cup.tile([128, KC, BL], bf16, tag="u")
                            elif s == R - 1:
                                u_nxt = u_st
                            else:
                                u_nxt = cup.tile([128, KC, BL], bf16, tag="u")
                            for half in range(2):
                                h0 = 2 * half
                                for kg in range(2):
                                    for G in range(NG):
                                        for ms in (h0, h0 + 1):
                                            m = KC * G + ms
                                            for k in (2 * kg, 2 * kg + 1):
                                                nc.tensor.matmul(
                                                    pslice(G)[:, ms, :],
                                                    w5_sb[:, m, k, :],
                                                    u_cur[:, k, :],
                                                    start=False,
                                                    stop=(k == KC - 1),
                                                    skip_group_check=True)
                                hsl = slice(h0, h0 + 2)
                                sif = csp.tile([128, 2, 2, BL], f32,
                                               tag=f"sif{half}")
                                mg = csp.tile([128, 2, BL], f32,
                                              tag=f"mg{half}")
                                og = csp.tile([128, 2, BL], f32,
                                              tag=f"og{half}")
                                hg = csp.tile([128, 2, BL], f32,
                                              tag=f"hg{half}")
                                nc.scalar.activation(sif[:],
                                                     psA[:, 0:2, hsl, :],
                                                     AF.Sigmoid)
                                nc.scalar.activation(mg[:], psA[:, 2, hsl, :],
                                                     AF.Tanh)
                                nc.scalar.activation(og[:], psB[:, 0, hsl, :],
                                                     AF.Sigmoid)
                                t1 = csp.tile([128, 2, BL], f32,
                                              tag=f"t1{half}")
                                nc.vector.tensor_mul(t1[:], sif[:, 0, :, :],
                                                     mg[:])
                                t2 = csp.tile([128, 2, BL], f32,
                                              tag=f"t2{half}")
                                nc.vector.tensor_mul(t2[:], sif[:, 1, :, :],
                                                     c_st[:, hsl, :])
                                nc.vector.tensor_add(c_st[:, hsl, :], t1[:],
                                                     t2[:])
                                tch = csp.tile([128, 2, BL], f32,
                                               tag=f"tch{half}")
                                nc.scalar.activation(tch[:], c_st[:, hsl, :],
                                                     AF.Tanh)
                                opv = csp.tile([128, 2, BL], f32,
                                               tag=f"opv{half}")
                                nc.vector.tensor_mul(opv[:], og[:], tch[:])
                                dv = csp.tile([128, 2, BL], f32,
                                              tag=f"dv{half}")
                                nc.vector.tensor_sub(dv[:], opv[:],
                                                     pblk[:, s, hsl, :])
                                nc.scalar.activation(hg[:], psB[:, 1, hsl, :],
                                                     AF.Sigmoid)
                                ev = csp.tile([128, 2, BL], f32,
                                              tag=f"ev{half}")
                                nc.vector.tensor_mul(ev[:], hg[:], dv[:])
                                nc.vector.tensor_add(u_nxt[:, hsl, :],
                                                     xp[:, s, hsl, :], ev[:])
                                nc.vector.tensor_add(ring[:, s, hsl, :],
                                                     ev[:],
                                                     pblk[:, s, hsl, :])
                            u_cur = u_st if cut_chain else u_nxt
                            continue
                        for G in range(NG):
                            for ms in range(KC):
                                m = KC * G + ms
                                for k in range(KC):
                                    nc.tensor.matmul(
                                        pslice(G)[:, ms, :],
                                        w5_sb[:, m, k, :],
                                        u_cur[:, k, :],
                                        start=False, stop=(k == KC - 1),
                                        skip_group_check=True)
                        sif = csp.tile([128, 2, KC, BL], f32, tag="sif")
                        mg = csp.tile([128, KC, BL], f32, tag="mg")
                        og = csp.tile([128, KC, BL], f32, tag="og")
                        hg = csp.tile([128, KC, BL], f32, tag="hg")
                        nc.scalar.activation(sif[:], psA[:, 0:2, :, :], AF.Sigmoid)
                        nc.scalar.activation(mg[:], psA[:, 2, :, :], AF.Tanh)
                        nc.scalar.activation(og[:], psB[:, 0, :, :], AF.Sigmoid)
                        t1 = csp.tile([128, KC, BL], f32, tag="t1")
                        nc.vector.tensor_mul(t1[:], sif[:, 0, :, :], mg[:])
                        t2 = csp.tile([128, KC, BL], f32, tag="t2")
                        nc.vector.tensor_mul(t2[:], sif[:, 1, :, :], c_st[:])
                        nc.vector.tensor_add(c_st[:], t1[:], t2[:])
                        tch = csp.tile([128, KC, BL], f32, tag="tch")
                        nc.scalar.activation(tch[:], c_st[:], AF.Tanh)
                        opv = csp.tile([128, KC, BL], f32, tag="opv")
                        nc.vector.tensor_mul(opv[:], og[:], tch[:])
                        dv = csp.tile([128, KC, BL], f32, tag="dv")
                        nc.vector.tensor_sub(dv[:], opv[:], pblk[:, s, :, :])
                        ev = csp.tile([128, KC, BL], f32, tag="ev")
                        if cut_chain:
                            u_nxt = cup.tile([128, KC, BL], bf16, tag="u")
                        elif s == R - 1:
                            u_nxt = u_st
                        else:
                            u_nxt = cup.tile([128, KC, BL], bf16, tag="u")
                        if tailv2:
                            # merged tail: one ACT + two DVE ops instead of
                            # 4x(ACT+2 DVE) — cuts end-of-step ACT FIFO
                            # congestion (each ACT costs ~(N+352)/1.2 ns).
                            nc.scalar.activation(hg[:], psB[:, 1, :, :],
                                                 AF.Sigmoid)
                            nc.vector.tensor_mul(ev[:], hg[:], dv[:])
                            nc.vector.tensor_add(u_nxt[:], xp[:, s, :, :],
                                                 ev[:])
                        else:
                            # hw-gate sigmoid, ev, and u are chunked per
                            # h-chunk so u[:, 0, :] lands as soon as psg4's
                            # ms=0 column stops; the next step's k=0 matmuls
                            # start while ms=1..3 of this step still drain
                            # through the PE.
                            for ms in range(KC):
                                nc.scalar.activation(hg[:, ms, :],
                                                     psB[:, 1, ms, :],
                                                     AF.Sigmoid)
                                nc.vector.tensor_mul(ev[:, ms, :],
                                                     hg[:, ms, :],
                                                     dv[:, ms, :])
                                nc.vector.tensor_add(u_nxt[:, ms, :],
                                                     xp[:, s, ms, :],
                                                     ev[:, ms, :])
                        nc.vector.tensor_add(ring[:, s, :, :], ev[:],
                                             pblk[:, s, :, :])
                        u_cur = u_st if cut_chain else u_nxt

                    nc.sync.dma_start(outT[:, ds(i * R * CW, R * CW)], ring[:])

    nc.compile()
    return nc


def _get_module(T, R, iters=None, outer_reps=1, staggered=False, hints=(),
                cut_chain=False, deep_bufs=False, wdt="bf16", psum2=False,
                tailv2=False, tailv3=False, bias2=False):
    key = (T, R, iters, outer_reps, staggered, tuple(hints), cut_chain,
           deep_bufs, wdt, psum2, tailv2, tailv3, bias2)
    if key not in _CACHE:
        _CACHE[key] = _build(T, R, iters, outer_reps, staggered, hints,
                             cut_chain, deep_bufs, wdt, psum2, tailv2, tailv3,
                             bias2)
    return _CACHE[key]


def _make_in_maps(x, W_in, b_in, R, wdt="bf16"):
    B, T, D = x.shape
    Tp = ((T + R - 1) // R) * R
    Wt = W_in.T  # [D, 6H]
    W5 = Wt[:, :NG * H]
    W6 = Wt[:, NG * H:]
    w5_np = ml_dtypes.float8_e4m3fn if wdt in ("fp8", "fp8a") else ml_dtypes.bfloat16
    b_np = ml_dtypes.float8_e4m3fn if wdt == "fp8a" else ml_dtypes.bfloat16
    w5_arr = np.ascontiguousarray(
        W5.reshape(KC, 128, M5, 128).transpose(1, 2, 0, 3)
        .reshape(128, M5 * KC * 128)).astype(w5_np)
    w6_arr = np.ascontiguousarray(
        W6.reshape(KC, 128, KC, 128).transpose(1, 2, 0, 3)
        .reshape(128, KC * KC * 128)).astype(np.float32)
    b5k_arr = np.ascontiguousarray((2.0 * b_in[:NG * H]).reshape(M5, 128)
                                   ).astype(b_np)
    sel_arr = np.zeros((KC, CW), b_np)
    for k in range(KC):
        sel_arr[k, BL * k:BL * k + BL] = 1.0
    # merged-bias indicators (bias2): rows (g,ms) -> cols (g,ms,b)
    selA_arr = np.zeros((3 * KC, 3 * KC * BL), b_np)
    for r in range(3 * KC):
        selA_arr[r, BL * r:BL * r + BL] = 1.0
    selB_arr = np.zeros((2 * KC, 2 * KC * BL), b_np)
    for r in range(2 * KC):
        selB_arr[r, BL * r:BL * r + BL] = 1.0
    b6_arr = np.ascontiguousarray(b_in[NG * H:].reshape(KC, 128).T
                                  ).astype(np.float32)
    in_maps = []
    for c in range(NCORES):
        xs = x[BL * c:BL * (c + 1)]  # [BL, T, D]
        xTa = np.zeros((128, (Tp + R) * CW), np.float32)
        xTa[:, :T * CW] = (xs.reshape(BL, T, KC, 128).transpose(3, 1, 2, 0)
                           .reshape(128, T * CW))
        in_maps.append({"xT": xTa, "w5": w5_arr, "w6": w6_arr,
                        "b5k": b5k_arr, "sel": sel_arr, "b6": b6_arr,
                        "selA": selA_arr, "selB": selB_arr})
    return in_maps


WDT = "fp8a"
# Best measured configuration (see perf notes): tailv3 = half-split step with
# 4-group matmul emission + per-half tails + double-buffered PSUM; removes
# the end-of-step dependency stall entirely (measured 4962 vs 6052 ns/step
# in-batch vs tailv2, bit-identical output).
BEST = dict(deep_bufs=True, tailv3=True, psum2=True)


def kernel(x, lengths, W_in, b_in):
    from concourse import bass_utils

    x = np.asarray(x, dtype=np.float32)
    lengths = np.asarray(lengths).astype(np.int64)
    W_in = np.asarray(W_in, dtype=np.float32)
    b_in = np.asarray(b_in, dtype=np.float32)
    B, T, D = x.shape
    R = 32
    nc = _get_module(T, R, wdt=WDT, **BEST)
    in_maps = _make_in_maps(x, W_in, b_in, R, wdt=WDT)
    res = bass_utils.run_bass_kernel_spmd(nc, in_maps,
                                          core_ids=list(range(NCORES)))
    out = np.zeros((B, T, D), np.float32)
    Tp = ((T + R - 1) // R) * R
    for c in range(NCORES):
        oT = np.asarray(res.results[c]["outT"])[:, :T * CW]
        oc = (oT.reshape(128, T, KC, BL).transpose(3, 1, 2, 0)
              .reshape(BL, T, D))
        out[BL * c:BL * (c + 1)] = oc
    mask = np.arange(T)[None, :] < lengths[:, None]
    out *= mask[:, :, None].astype(np.float32)
    return out

